# revision 3
# baseline (speedup 1.0000x reference)
"""AMICO ADMM solver on 8 Trainium2 NeuronCores.

Problem: X = argmin ||Y^T - A x||^2 + lam*||x||_1 s.t. x >= 0, solved with
max_iter ADMM steps (rho=1, lam=0.1) exactly as in the reference scan.

Algebraic reduction (tracking only v = x + u):
    v_1 = G                      with G  = Minv @ A^T @ Y^T
    for i = 2..N:
        w   = |v - t|            (t = lam/rho)
        S   = min(v, t) + Gb     (Gb = G - t * Minv @ 1)
        v'  = Minv @ w + S
    output x_N = Minv @ w_{N-1} + Gb

since z = relu(v - t), u' = v - z = min(v, t), and z - u' = |v - t| - t.
The constant -t*Minv@1 and the A^T Y^T term are folded into a single
"augmented" matmul: Gb = Ht_aug^T @ Yt_aug where Ht_aug carries A@Minv plus a
bias row (-t * rowsum(Minv)) and Yt_aug carries Y^T plus a row of ones.

Sharding: data-parallel over voxels (B=4096 -> 512 per core); A-derived
matrices (Minv, Ht_aug) replicated; no cross-core communication.

Device work per core per iteration: 16 fp32r matmuls [128x128 @ 128x512]
(PE), one Abs activation (ACT), and two fused scalar_tensor_tensor ops (DVE)
over [512, 512] state kept entirely in SBUF/PSUM.
"""

import numpy as np

B_VOX = 4096
M_MEAS = 256
K_ATOMS = 512
P = 128
N_CORES = 8
BS = B_VOX // N_CORES  # 512 voxels per core
KB = K_ATOMS // P  # 4 chunks of the contraction/output dim
LAM = 0.1
RHO = 1.0
THR = LAM / RHO

_NC_CACHE = {}


def _build(niter):
    import concourse.mybir as mybir
    import concourse.tile as tile
    from concourse import bacc

    f32 = mybir.dt.float32
    f32r = mybir.dt.float32r
    Alu = mybir.AluOpType
    Act = mybir.ActivationFunctionType

    nc = bacc.Bacc(None, target_bir_lowering=False)
    yt = nc.declare_dram_parameter("Yt", [3 * P, BS], f32, isOutput=False)
    ht = nc.declare_dram_parameter("Ht", [3 * P, K_ATOMS], f32, isOutput=False)
    mi = nc.declare_dram_parameter("Mi", [K_ATOMS, K_ATOMS], f32, isOutput=False)
    rs = nc.declare_dram_parameter("rs", [P, KB], f32, isOutput=False)
    out = nc.declare_dram_parameter("out", [K_ATOMS, BS], f32, isOutput=True)

    with tile.TileContext(nc) as tc:
        with (
            tc.tile_pool(name="const", bufs=1) as cpool,
            tc.tile_pool(name="v", bufs=8) as vpool,
            tc.tile_pool(name="w", bufs=12) as wpool,
            tc.tile_pool(name="s", bufs=8) as spool,
            tc.tile_pool(name="o", bufs=4) as opool,
            tc.tile_pool(name="psum", bufs=8, space="PSUM") as ppool,
        ):
            nb = cpool.tile([P, 1], f32)
            nc.vector.memset(nb[:], -THR)
            mi_sb = cpool.tile([P, KB, K_ATOMS], f32r)
            nc.gpsimd.dma_start(mi_sb[:], mi.rearrange("(kb p) m -> p kb m", p=P))
            ht_sb = cpool.tile([P, 3, K_ATOMS], f32r)
            nc.gpsimd.dma_start(ht_sb[:], ht.rearrange("(kb p) a -> p kb a", p=P))
            yt_sb = cpool.tile([P, 3, BS], f32r)
            nc.gpsimd.dma_start(yt_sb[:], yt.rearrange("(kb p) b -> p kb b", p=P))
            rs_sb = cpool.tile([P, KB], f32)
            nc.sync.dma_start(rs_sb[:], rs[:])
            gb_sb = cpool.tile([P, KB, BS], f32)

            outr = out.rearrange("(mb p) n -> mb p n", p=P)

            w_cur = [None] * KB
            s_cur = [None] * KB

            # ---- iteration 1: Gb = Ht_aug^T @ Yt_aug ----
            pgs = [ppool.tile([P, BS], f32, tag="pp", name=f"pg{m}") for m in range(KB)]
            for kb in range(3):
                for m in range(KB):
                    nc.tensor.matmul(
                        pgs[m][:],
                        lhsT=ht_sb[:, kb, m * P : (m + 1) * P],
                        rhs=yt_sb[:, kb, :],
                        start=(kb == 0),
                        stop=(kb == 2),
                    )
            if niter == 1:
                for m in range(KB):
                    xm = opool.tile([P, BS], f32, tag="x")
                    nc.vector.tensor_scalar_add(xm[:], pgs[m][:], rs_sb[:, m : m + 1])
                    nc.sync.dma_start(outr[m], xm[:])
            else:
                for m in range(KB):
                    # Gb to SBUF (needed every iteration)
                    nc.scalar.activation(gb_sb[:, m, :], pgs[m][:], Act.Copy)
                    # v_1 = G = Gb + t*rowsum(Minv)
                    vm = vpool.tile([P, BS], f32, tag="v")
                    nc.vector.tensor_scalar_add(vm[:], pgs[m][:], rs_sb[:, m : m + 1])
                    wm = wpool.tile([P, BS], f32r, tag="w")
                    nc.scalar.activation(wm[:], vm[:], Act.Abs, bias=nb[:, 0:1])
                    sm = spool.tile([P, BS], f32, tag="s")
                    nc.vector.scalar_tensor_tensor(
                        sm[:], vm[:], THR, gb_sb[:, m, :], Alu.min, Alu.add
                    )
                    w_cur[m], s_cur[m] = wm, sm

            # ---- iterations 2..niter ----
            for it in range(2, niter + 1):
                last = it == niter
                pps = [ppool.tile([P, BS], f32, tag="pp", name=f"pp{it}_{m}") for m in range(KB)]
                for kb in range(KB):
                    for m in range(KB):
                        nc.tensor.matmul(
                            pps[m][:],
                            lhsT=mi_sb[:, kb, m * P : (m + 1) * P],
                            rhs=w_cur[kb][:],
                            start=(kb == 0),
                            stop=(kb == KB - 1),
                        )
                neww = [None] * KB
                news = [None] * KB
                for m in range(KB):
                    if last:
                        xm = opool.tile([P, BS], f32, tag="x")
                        nc.vector.scalar_tensor_tensor(
                            xm[:], pps[m][:], 0.0, gb_sb[:, m, :], Alu.bypass, Alu.add
                        )
                        nc.sync.dma_start(outr[m], xm[:])
                    else:
                        vm = vpool.tile([P, BS], f32, tag="v")
                        nc.vector.scalar_tensor_tensor(
                            vm[:], pps[m][:], 0.0, s_cur[m][:], Alu.bypass, Alu.add
                        )
                        wm = wpool.tile([P, BS], f32r, tag="w")
                        nc.scalar.activation(wm[:], vm[:], Act.Abs, bias=nb[:, 0:1])
                        sm = spool.tile([P, BS], f32, tag="s")
                        nc.vector.scalar_tensor_tensor(
                            sm[:], vm[:], THR, gb_sb[:, m, :], Alu.min, Alu.add
                        )
                        neww[m], news[m] = wm, sm
                if not last:
                    w_cur, s_cur = neww, news

    nc.finalize()
    return nc


def _get_nc(niter):
    if niter not in _NC_CACHE:
        _NC_CACHE[niter] = _build(niter)
    return _NC_CACHE[niter]


def _prep_in_maps(Y, A):
    """Host precompute of the A-derived (voxel-independent) factor matrices,
    in float64: the inverse replaces the reference's Cholesky solve. Shards Y
    over voxels (transposed, with the augmented ones-row appended)."""
    A64 = A.astype(np.float64)
    LHS = A64.T @ A64 + RHO * np.eye(K_ATOMS)
    Minv = np.linalg.inv(LHS)
    Minv = (Minv + Minv.T) / 2
    Hm = A64 @ Minv  # [M, K]
    rsum = Minv.sum(axis=1)

    Ht = np.zeros((3 * P, K_ATOMS), np.float32)
    Ht[:M_MEAS] = Hm.astype(np.float32)
    Ht[M_MEAS] = (-THR * rsum).astype(np.float32)
    Mi = Minv.astype(np.float32)
    rs = np.ascontiguousarray((THR * rsum).astype(np.float32).reshape(KB, P).T)

    in_maps = []
    for c in range(N_CORES):
        Yt = np.zeros((3 * P, BS), np.float32)
        Yt[:M_MEAS] = Y[c * BS : (c + 1) * BS, :].T
        Yt[M_MEAS] = 1.0
        in_maps.append(
            {"Yt": np.ascontiguousarray(Yt), "Ht": Ht, "Mi": Mi, "rs": rs}
        )
    return in_maps


def kernel(Y, A, max_iter):
    from concourse.bass_utils import run_bass_kernel_spmd

    Y = np.ascontiguousarray(np.asarray(Y, dtype=np.float32))
    A = np.ascontiguousarray(np.asarray(A, dtype=np.float32))
    niter = int(max_iter)
    assert Y.shape == (B_VOX, M_MEAS) and A.shape == (M_MEAS, K_ATOMS)
    assert niter >= 1

    in_maps = _prep_in_maps(Y, A)
    nc = _get_nc(niter)
    res = run_bass_kernel_spmd(nc, in_maps, core_ids=list(range(N_CORES)))

    outp = np.empty((B_VOX, K_ATOMS), np.float32)
    for c in range(N_CORES):
        outp[c * BS : (c + 1) * BS] = res.results[c]["out"].T
    return outp
